# revision 17
# baseline (speedup 1.0000x reference)
"""Trainium2 Bass kernel for MockTriangleMultiplication (outgoing triangle update).

Full-input contract: kernel(**inputs) takes the unsharded reference inputs and
returns the full [1, 512, 512, 128] output. Internally shards the first N (row)
axis of z/mask across 8 NeuronCores (sequence parallel); b rows are AllGathered
(FastFold-style dynamic-axial parallelism for the outgoing einsum).

Host/dispatch path is optimized for the axon tunnel (~47 MB/s shared by both
directions, no parallel-stream scaling):
  - z is uploaded as int8 with a per-token absmax scale folded in on the host;
    LayerNorm is scale-invariant per token, so the device needs no dequant
    scales at all (34MB instead of 134MB f32)
  - the device returns DELTA = out - z as int8 plus a per-token f32 scale
    (34.5MB down); the dequant + residual add happens on the host in f32,
    preserving full z precision
  - output zero-buffers live on-device (created once), not uploaded per call
  - the jitted shard_map callable is built once and cached across calls
    (the stock run_bass_kernel_spmd re-jits + re-uploads everything per call)
  - host-side scratch buffers are preallocated once to avoid 134MB of page
    faults per call

Device pipeline per core (rows r in its 64-row shard):
  phase 1: z -> LN -> transpose -> 4 projections -> sigmoid gates (+mask)
           -> a^T, b^T stored [c, row, col] in bf16
  AllGather b^T over 8 cores -> b_all [rank, c, k_loc, j]
  phase 2: per channel c: OUT_c[i_shard, j] = A_c[i_shard, :] @ B_c  (PSUM k-acc)
  phase 3: delta = OUT @ W_z + b_z  (token-major matmul), bf16 out

LayerNorm affine (ln_w, ln_b) is folded into the projection weights/biases on
the host, so the device does plain whitening only.
"""

import os

import numpy as np
import ml_dtypes

import concourse.bass as bass
import concourse.bacc as bacc
import concourse.tile as tile
import concourse.mybir as mybir
import concourse.bass2jax as bass2jax
import concourse.masks as masks

F32 = mybir.dt.float32
BF16 = mybir.dt.bfloat16
I8 = mybir.dt.int8
AF = mybir.ActivationFunctionType
OP = mybir.AluOpType

R = int(os.environ.get("K_R", "8"))      # cores
N = int(os.environ.get("K_N", "512"))    # sequence
C = 128                                  # channels (c_z == c_hid)
SH = int(os.environ.get("K_SH", str(N // R)))  # rows per core
T4 = N // C    # 128-token tiles per row (4)
NQ = N // C    # k-chunks of 128 in the einsum
OCT = 8        # channels per phase-2 block

MASK_MODE = os.environ.get("K_MASK", 'pe')

_CACHE = {}


def _phase1(tc, cst, z_rows, a_loc, b_loc):
    nc = tc.nc
    with (
        tc.tile_pool(name="p1", bufs=3) as p1,
        tc.tile_pool(name="p1st", bufs=3) as p1st,
        tc.tile_pool(name="ps_zt", bufs=2, space="PSUM") as ps_zt,
        tc.tile_pool(name="ps_proj", bufs=1, space="PSUM") as ps_proj,
        tc.tile_pool(name="ps_mask", bufs=1, space="PSUM") as ps_mask,
    ):
        for r in range(SH):
            z_i8 = p1.tile([C, N], I8, tag="z_i8")
            # [tok, (t, c)] <- z_rows[r] viewed (t p) c -> p t c
            nc.gpsimd.dma_start(
                z_i8[:].rearrange("p (t c) -> p t c", t=T4),
                z_rows[r].rearrange("(t p) c -> p t c", p=C),
            )
            # int8 -> bf16; values are per-token scaled but LN whitening is
            # scale-invariant per token, so no dequant scale is needed
            z_sb = p1.tile([C, N], BF16, tag="z_sb")
            nc.vector.tensor_copy(z_sb[:], z_i8[:])
            mu4 = p1st.tile([C, T4], F32, tag="mu4")
            ssq4 = p1st.tile([C, T4], F32, tag="ssq4")
            sq_scr = p1st.tile([C, C], BF16, tag="sq_scr")
            for t in range(T4):
                zt = z_sb[:, t * C:(t + 1) * C]
                nc.vector.tensor_reduce(mu4[:, t:t + 1], zt,
                                        mybir.AxisListType.X, OP.add)
                nc.scalar.activation(sq_scr[:], zt, AF.Square,
                                     accum_out=ssq4[:, t:t + 1])
            nmu4 = p1st.tile([C, T4], F32, tag="nmu4")
            nc.vector.tensor_scalar_mul(nmu4[:], mu4[:], -1.0 / C)
            mu2 = p1st.tile([C, T4], F32, tag="mu2")
            nc.vector.tensor_tensor(mu2[:], nmu4[:], nmu4[:], OP.mult)
            var4 = p1st.tile([C, T4], F32, tag="var4")
            nc.vector.tensor_scalar_mul(var4[:], ssq4[:], 1.0 / C)
            var4b = p1st.tile([C, T4], F32, tag="var4b")
            nc.vector.tensor_tensor(var4b[:], var4[:], mu2[:], OP.subtract)
            std4 = p1st.tile([C, T4], F32, tag="std4")
            nc.scalar.activation(std4[:], var4b[:], AF.Sqrt,
                                 bias=cst['eps'][:])
            rstd4 = p1st.tile([C, T4], F32, tag="rstd4")
            nc.vector.reciprocal(rstd4[:], std4[:])

            zn_sb = p1.tile([C, N], BF16, tag="zn_sb")
            zT_ps = ps_zt.tile([C, N], BF16, tag="zT_ps")
            for t in range(T4):
                zt = z_sb[:, t * C:(t + 1) * C]
                znt = zn_sb[:, t * C:(t + 1) * C]
                nc.vector.tensor_scalar(
                    znt, zt, nmu4[:, t:t + 1], rstd4[:, t:t + 1],
                    OP.add, OP.mult)
                nc.tensor.transpose(zT_ps[:, t * C:(t + 1) * C], znt,
                                    cst['ident'][:])
            zT_sb = p1.tile([C, N], BF16, tag="zT_sb")
            nc.vector.tensor_copy(zT_sb[:], zT_ps[:])

            pap = ps_proj.tile([C, N], F32, tag="pap")
            pag = ps_proj.tile([C, N], F32, tag="pag")
            pbp = ps_proj.tile([C, N], F32, tag="pbp")
            pbg = ps_proj.tile([C, N], F32, tag="pbg")
            nc.tensor.matmul(pap[:], cst['wap'][:], zT_sb[:], start=True, stop=True)
            nc.tensor.matmul(pag[:], cst['wag'][:], zT_sb[:], start=True, stop=True)
            nc.tensor.matmul(pbp[:], cst['wbp'][:], zT_sb[:], start=True, stop=True)
            nc.tensor.matmul(pbg[:], cst['wbg'][:], zT_sb[:], start=True, stop=True)

            pa_sb = p1.tile([C, N], BF16, tag="pa_sb")
            pb_sb = p1.tile([C, N], BF16, tag="pb_sb")
            ga_sb = p1.tile([C, N], BF16, tag="ga_sb")
            gb_sb = p1.tile([C, N], BF16, tag="gb_sb")
            nc.vector.tensor_scalar_add(pa_sb[:], pap[:], cst['bap'][:])
            nc.scalar.activation(pb_sb[:], pbp[:], AF.Identity,
                                 bias=cst['bbp'][:])
            nc.scalar.activation(ga_sb[:], pag[:], AF.Sigmoid,
                                 bias=cst['bag'][:])
            nc.scalar.activation(gb_sb[:], pbg[:], AF.Sigmoid,
                                 bias=cst['bbg'][:])

            a1 = p1.tile([C, N], BF16, tag="a1")
            b1 = p1.tile([C, N], BF16, tag="b1")
            nc.vector.tensor_tensor(a1[:], pa_sb[:], ga_sb[:], OP.mult)
            nc.vector.tensor_tensor(b1[:], pb_sb[:], gb_sb[:], OP.mult)
            if MASK_MODE != 'skip':
                # mask row broadcast to 128 partitions via K=1 ones-matmul
                mask_ps = ps_mask.tile([C, N], F32, tag="mask_ps")
                nc.tensor.matmul(mask_ps[:], cst['ones1'][:],
                                 cst['mask'][:, r * N:(r + 1) * N],
                                 start=True, stop=True)
                mask_sb = p1.tile([C, N], BF16, tag="mask_sb")
                nc.scalar.copy(mask_sb[:], mask_ps[:])
                am = p1.tile([C, N], BF16, tag="am")
                bm = p1.tile([C, N], BF16, tag="bm")
                nc.vector.tensor_tensor(am[:], a1[:], mask_sb[:], OP.mult)
                nc.vector.tensor_tensor(bm[:], b1[:], mask_sb[:], OP.mult)
            else:
                am, bm = a1, b1
            nc.sync.dma_start(a_loc[:, r, :], am[:])
            nc.sync.dma_start(b_loc[:, r, :], bm[:])


def _phase2(tc, a_loc, b_all, o_mid):
    nc = tc.nc
    with (
        tc.tile_pool(name="p2a", bufs=2) as p2a,
        tc.tile_pool(name="p2b", bufs=2) as p2b,
        tc.tile_pool(name="p2o", bufs=3) as p2o,
        tc.tile_pool(name="ps_o", bufs=2, space="PSUM") as ps_o_pool,
    ):
        b_all_v = b_all[:].rearrange("(r c) k j -> r c k j", r=R)
        a_2d = a_loc[:].rearrange("c i k -> (c i) k")
        for oc in range(C // OCT):
            aT_t = []
            for q in range(NQ):
                at = p2a.tile([C, OCT * SH], BF16, tag=f"aT{q}")
                # src: a_loc[c-octet, :, k-chunk] as [(c i), k] 2D
                nc.sync.dma_start_transpose(
                    at[:],
                    a_2d[OCT * oc * SH:OCT * (oc + 1) * SH,
                         C * q:C * (q + 1)],
                )
                aT_t.append(at)
            RK = C // SH  # ranks per 128-row k-chunk
            b_t = []
            for q in range(NQ):
                bt = p2b.tile([C, OCT * N], BF16, tag=f"bT{q}")
                for rr in range(RK):
                    nc.sync.dma_start(
                        bt[rr * SH:(rr + 1) * SH, :].rearrange(
                            "k (c j) -> k c j", c=OCT),
                        b_all_v[RK * q + rr,
                                OCT * oc:OCT * (oc + 1), :, :].rearrange(
                            "c k j -> k c j"),
                    )
                b_t.append(bt)
            for ci in range(0, OCT, 2):
                o_sb = p2o.tile([SH, 2 * N], BF16, tag="o_sb")
                for cj in range(2):
                    ps_o = ps_o_pool.tile([SH, N], F32, tag="ps_o")
                    for q in range(NQ):
                        nc.tensor.matmul(
                            ps_o[:],
                            aT_t[q][:, (ci + cj) * SH:(ci + cj + 1) * SH],
                            b_t[q][:, (ci + cj) * N:(ci + cj + 1) * N],
                            start=(q == 0), stop=(q == NQ - 1))
                    nc.vector.tensor_copy(o_sb[:, cj * N:(cj + 1) * N],
                                          ps_o[:])
                c0 = OCT * oc + ci
                nc.sync.dma_start(
                    o_mid[c0:c0 + 2, :, :].rearrange("c k j -> k c j"),
                    o_sb[:].rearrange("k (c j) -> k c j", c=2))


def _phase3(tc, cst, o_mid, out_rows, scale_rows):
    nc = tc.nc
    with (
        tc.tile_pool(name="p3", bufs=3) as p3,
        tc.tile_pool(name="ps_f", bufs=4, space="PSUM") as ps_f_pool,
    ):
        for r in range(SH):
            oT_sb = p3.tile([C, N], BF16, tag="oT_sb")
            nc.sync.dma_start(oT_sb[:], o_mid[:, r, :])
            d32 = p3.tile([C, N], F32, tag="d32")
            am = p3.tile([C, T4], F32, tag="am")
            for t in range(T4):
                ps_f = ps_f_pool.tile([C, C], F32, tag="ps_f")
                nc.tensor.matmul(ps_f[:], oT_sb[:, t * C:(t + 1) * C],
                                 cst['wz'][:], start=True, stop=True)
                nc.vector.tensor_tensor(
                    d32[:, t * C:(t + 1) * C], ps_f[:],
                    cst['bzbc'][:], OP.add)
                nc.vector.tensor_reduce(
                    am[:, t:t + 1], d32[:, t * C:(t + 1) * C],
                    mybir.AxisListType.X, OP.max, apply_absolute_value=True)
            # per-token int8 quantization: q = d * 126/absmax, sc = absmax/126
            amc = p3.tile([C, T4], F32, tag="amc")
            nc.vector.tensor_scalar_max(amc[:], am[:], 1e-12)
            rs = p3.tile([C, T4], F32, tag="rs")
            nc.vector.reciprocal(rs[:], amc[:])
            rs126 = p3.tile([C, T4], F32, tag="rs126")
            nc.vector.tensor_scalar_mul(rs126[:], rs[:], 126.0)
            sc = p3.tile([C, T4], F32, tag="sc")
            nc.vector.tensor_scalar_mul(sc[:], amc[:], 1.0 / 126.0)
            q_i8 = p3.tile([C, N], I8, tag="q_i8")
            for t in range(T4):
                nc.vector.tensor_scalar_mul(
                    q_i8[:, t * C:(t + 1) * C], d32[:, t * C:(t + 1) * C],
                    rs126[:, t:t + 1])
            nc.sync.dma_start(
                out_rows[r].rearrange("(t p) c -> p t c", p=C),
                q_i8[:].rearrange("p (t c) -> p t c", t=T4))
            nc.sync.dma_start(
                scale_rows[r].rearrange("(t p) -> p t", p=C),
                sc[:])


def build():
    if 'nc' in _CACHE:
        return _CACHE['nc']
    nc = bacc.Bacc("TRN2", target_bir_lowering=False, debug=False,
                   num_devices=R)

    z_rows = nc.dram_tensor("z_rows", [SH, N, C], I8, kind="ExternalInput")
    mask_rows = nc.dram_tensor("mask_rows", [SH, N], F32, kind="ExternalInput")
    w_in = {}
    for nm in ("w_ap", "w_ag", "w_bp", "w_bg", "w_z"):
        w_in[nm] = nc.dram_tensor(nm, [C, C], BF16, kind="ExternalInput")
    b_in = {}
    for nm in ("b_ap", "b_ag", "b_bp", "b_bg"):
        b_in[nm] = nc.dram_tensor(nm, [C, 1], F32, kind="ExternalInput")
    bz_bc = nc.dram_tensor("bz_bc", [C, C], F32, kind="ExternalInput")
    out_rows = nc.dram_tensor("out_rows", [SH, N, C], I8,
                              kind="ExternalOutput")
    scale_rows = nc.dram_tensor("scale_rows", [SH, N], F32,
                                kind="ExternalOutput")

    with tile.TileContext(nc) as tc:
        with (
            tc.tile_pool(name="consts", bufs=1) as cpool,
            tc.tile_pool(name="dram", bufs=1, space="DRAM") as dram,
        ):
            cst = {}
            ident = cpool.tile([C, C], BF16)
            masks.make_identity(nc, ident[:])
            cst['ident'] = ident
            for nm, key in (("w_ap", 'wap'), ("w_ag", 'wag'),
                            ("w_bp", 'wbp'), ("w_bg", 'wbg'), ("w_z", 'wz')):
                t = cpool.tile([C, C], BF16, tag=f"c_{key}")
                nc.sync.dma_start(t[:], w_in[nm][:])
                cst[key] = t
            for nm, key in (("b_ap", 'bap'), ("b_ag", 'bag'),
                            ("b_bp", 'bbp'), ("b_bg", 'bbg')):
                t = cpool.tile([C, 1], F32, tag=f"c_{key}")
                nc.sync.dma_start(t[:], b_in[nm][:])
                cst[key] = t
            bzbc = cpool.tile([C, C], F32)
            nc.sync.dma_start(bzbc[:], bz_bc[:])
            cst['bzbc'] = bzbc
            # whole mask shard on partition 0, bf16 (for K=1 broadcast matmuls)
            mask_p0 = cpool.tile([1, SH * N], BF16)
            nc.gpsimd.dma_start(mask_p0[:],
                                mask_rows[:].rearrange("r n -> (r n)")
                                .unsqueeze(0))
            cst['mask'] = mask_p0
            ones1 = cpool.tile([1, C], BF16)
            nc.vector.memset(ones1[:], 1.0)
            cst['ones1'] = ones1
            eps = cpool.tile([C, 1], F32)
            nc.vector.memset(eps[:], 1e-5)
            cst['eps'] = eps

            a_loc = dram.tile([C, SH, N], BF16)      # [c, i_loc, k]
            b_loc = dram.tile([C, SH, N], BF16)      # [c, k_loc, j]
            b_all = dram.tile([R * C, SH, N], BF16,
                              addr_space="Shared")   # [(rank c), k_loc, j]
            o_mid = dram.tile([C, SH, N], BF16)      # [c, i_loc, j]

            _phase1(tc, cst, z_rows, a_loc, b_loc)
            nc.gpsimd.collective_compute(
                "AllGather", OP.bypass,
                replica_groups=[list(range(R))],
                ins=[b_loc[:].opt()],
                outs=[b_all[:].opt()],
            )
            _phase2(tc, a_loc, b_all, o_mid)
            _phase3(tc, cst, o_mid, out_rows, scale_rows)

    nc.compile()
    _CACHE['nc'] = nc
    return nc


def _get_runner():
    """Build (once) a cached jitted shard_map callable around the bass NEFF.

    Mirrors concourse.bass2jax.run_bass_via_pjrt's multi-core branch, except:
      - output zero-buffers are created on-device inside the body (the stock
        path uploads host zeros and donates them -- 67MB/call over the tunnel)
      - the jitted function is built once and cached, so repeat calls skip
        retracing/compilation and go straight to dispatch
    """
    if 'runner' in _CACHE:
        return _CACHE['runner']

    import jax
    import jax.numpy as jnp
    from jax.experimental.shard_map import shard_map
    from jax.sharding import Mesh, NamedSharding, PartitionSpec

    nc = build()
    bass2jax.install_neuronx_cc_hook()
    assert nc.dbg_addr is None

    partition_name = (nc.partition_id_tensor.name
                      if nc.partition_id_tensor else None)
    in_names = []
    out_names = []
    out_avals = []
    for alloc in nc.m.functions[0].allocations:
        if not isinstance(alloc, mybir.MemoryLocationSet):
            continue
        name = alloc.memorylocations[0].name
        if alloc.kind == "ExternalInput":
            if name != partition_name:
                in_names.append(name)
        elif alloc.kind == "ExternalOutput":
            out_names.append(name)
            out_avals.append(jax.core.ShapedArray(
                tuple(alloc.tensor_shape), mybir.dt.np(alloc.dtype)))
    n_params = len(in_names)
    n_outs = len(out_names)
    all_in_names = list(in_names) + list(out_names)
    if partition_name is not None:
        all_in_names.append(partition_name)

    def _body(*args):
        # args: kernel inputs followed by (device-resident) output zero bufs;
        # the hook requires every custom-call operand to be a jit parameter
        operands = list(args)
        if partition_name is not None:
            operands.append(bass2jax.partition_id_tensor())
        outs = bass2jax._bass_exec_p.bind(
            *operands,
            out_avals=tuple(out_avals),
            in_names=tuple(all_in_names),
            out_names=tuple(out_names),
            lowering_input_output_aliases=(),
            sim_require_finite=True,
            sim_require_nnan=True,
            nc=nc,
        )
        return tuple(outs)

    devices = jax.devices()[:R]
    assert len(devices) == R, f"need {R} devices, have {len(jax.devices())}"
    mesh = Mesh(np.asarray(devices), ("core",))
    in_specs = (PartitionSpec("core"),) * (n_params + n_outs)
    out_specs = (PartitionSpec("core"),) * n_outs
    sharded = jax.jit(shard_map(
        _body, mesh=mesh, in_specs=in_specs,
        out_specs=out_specs, check_rep=False))

    # output zero buffers: built on-device once, reused every call (the
    # kernel writes every output element, so stale contents are fine)
    shardings = [NamedSharding(mesh, PartitionSpec("core"))] * n_outs
    global_shapes = [(R * a.shape[0],) + tuple(a.shape[1:]) for a in out_avals]

    def _mk_zeros():
        return tuple(jnp.zeros(s, a.dtype)
                     for s, a in zip(global_shapes, out_avals))

    zeros_dev = jax.jit(_mk_zeros, out_shardings=tuple(shardings))()

    _CACHE['runner'] = (sharded, in_names, out_names, zeros_dev)
    return _CACHE['runner']


def _host_fns():
    """Fused single-pass CPU helpers (XLA fuses the multi-op numpy chains)."""
    if 'hostfns' not in _CACHE:
        import jax
        import jax.numpy as jnp
        from functools import partial

        @partial(jax.jit, backend='cpu')
        def quant(zf):
            am = jnp.max(jnp.abs(zf), axis=-1)
            rs = jnp.float32(126.0) / jnp.maximum(am, jnp.float32(1e-12))
            return jnp.rint(zf * rs[:, :, None]).astype(jnp.int8)

        @partial(jax.jit, backend='cpu')
        def dequant(q, sc, zf):
            return zf + q.astype(jnp.float32) * sc[:, :, None]

        _CACHE['hostfns'] = (quant, dequant)
    return _CACHE['hostfns']


def kernel(z, mask, ln_w, ln_b, W_ap, b_ap, W_ag, b_ag, W_bp, b_bp,
           W_bg, b_bg, W_z, b_z):
    z = np.asarray(z, dtype=np.float32)
    mask = np.asarray(mask, dtype=np.float32)
    ln_w = np.asarray(ln_w, np.float32)
    ln_b = np.asarray(ln_b, np.float32)
    bf = ml_dtypes.bfloat16

    def fold_w(W):
        return np.ascontiguousarray((ln_w[:, None] * np.asarray(W, np.float32))
                                    .astype(bf))

    def fold_b(b, W):
        return np.ascontiguousarray(
            (np.asarray(b, np.float32) + ln_b @ np.asarray(W, np.float32))
            .reshape(C, 1))

    weights = dict(
        w_ap=fold_w(W_ap), w_ag=fold_w(W_ag),
        w_bp=fold_w(W_bp), w_bg=fold_w(W_bg),
        b_ap=fold_b(b_ap, W_ap), b_ag=fold_b(b_ag, W_ag),
        b_bp=fold_b(b_bp, W_bp), b_bg=fold_b(b_bg, W_bg),
        w_z=np.ascontiguousarray(np.asarray(W_z, np.float32).astype(bf)),
        bz_bc=np.ascontiguousarray(
            np.broadcast_to(np.asarray(b_z, np.float32), (C, C))),
    )

    quant, dequant = _host_fns()
    zf = z.reshape(N, N, C)
    mask_full = mask.reshape(N, N)             # global (R*SH, N) concat

    # per-token int8 quantization of z (LN on device is scale-invariant,
    # so the scale never needs to leave the host)
    z_q = np.asarray(quant(zf))                # global (R*SH, N, C) concat

    sharded, in_names, out_names, zeros_dev = _get_runner()

    # global concat-along-axis-0 arrays, in in_names order
    global_ins = {
        'z_rows': z_q,
        'mask_rows': mask_full,
    }
    for k, v in weights.items():
        global_ins[k] = np.ascontiguousarray(
            np.tile(v, (R,) + (1,) * (v.ndim - 1)))

    args = [global_ins[name] for name in in_names]
    outs = sharded(*args, *zeros_dev)
    q = np.asarray(outs[out_names.index("out_rows")])      # (N, N, C) int8
    sc = np.asarray(outs[out_names.index("scale_rows")])   # (N, N) f32

    out = np.asarray(dequant(q, sc, zf))
    return out.reshape(1, N, N, C)


# revision 20
# speedup vs baseline: 1.1673x; 1.1673x over previous
"""Trainium2 Bass kernel for MockTriangleMultiplication (outgoing triangle update).

Full-input contract: kernel(**inputs) takes the unsharded reference inputs and
returns the full [1, 512, 512, 128] output. Internally shards the first N (row)
axis of z/mask across 8 NeuronCores (sequence parallel); b rows are AllGathered
(FastFold-style dynamic-axial parallelism for the outgoing einsum).

Host/dispatch path is optimized for the axon tunnel (~47 MB/s shared by both
directions, no parallel-stream scaling):
  - z is uploaded as int8 with a per-token absmax scale folded in on the host;
    LayerNorm is scale-invariant per token, so the device needs no dequant
    scales at all (34MB instead of 134MB f32)
  - the device returns DELTA = out - z as int8 plus a per-token f32 scale
    (34.5MB down); the dequant + residual add happens on the host in f32,
    preserving full z precision
  - output zero-buffers live on-device (created once), not uploaded per call
  - the jitted shard_map callable is built once and cached across calls
    (the stock run_bass_kernel_spmd re-jits + re-uploads everything per call)
  - host-side scratch buffers are preallocated once to avoid 134MB of page
    faults per call

Device pipeline per core (rows r in its 64-row shard):
  phase 1: z -> LN -> transpose -> 4 projections -> sigmoid gates (+mask)
           -> a^T, b^T stored [c, row, col] in bf16
  AllGather b^T over 8 cores -> b_all [rank, c, k_loc, j]
  phase 2: per channel c: OUT_c[i_shard, j] = A_c[i_shard, :] @ B_c  (PSUM k-acc)
  phase 3: delta = OUT @ W_z + b_z  (token-major matmul), bf16 out

LayerNorm affine (ln_w, ln_b) is folded into the projection weights/biases on
the host, so the device does plain whitening only.
"""

import os

import numpy as np
import ml_dtypes

import concourse.bass as bass
import concourse.bacc as bacc
import concourse.tile as tile
import concourse.mybir as mybir
import concourse.bass2jax as bass2jax
import concourse.masks as masks

F32 = mybir.dt.float32
BF16 = mybir.dt.bfloat16
I8 = mybir.dt.int8
AF = mybir.ActivationFunctionType
OP = mybir.AluOpType

R = int(os.environ.get("K_R", "8"))      # cores
N = int(os.environ.get("K_N", "512"))    # sequence
C = 128                                  # channels (c_z == c_hid)
SH = int(os.environ.get("K_SH", str(N // R)))  # rows per core
T4 = N // C    # 128-token tiles per row (4)
NQ = N // C    # k-chunks of 128 in the einsum
OCT = 8        # channels per phase-2 block

MASK_MODE = os.environ.get("K_MASK", 'pe')

_CACHE = {}


def _phase1(tc, cst, z_rows, a_loc, b_loc):
    nc = tc.nc
    with (
        tc.tile_pool(name="p1", bufs=3) as p1,
        tc.tile_pool(name="p1st", bufs=3) as p1st,
        tc.tile_pool(name="ps_zt", bufs=2, space="PSUM") as ps_zt,
        tc.tile_pool(name="ps_proj", bufs=1, space="PSUM") as ps_proj,
        tc.tile_pool(name="ps_mask", bufs=1, space="PSUM") as ps_mask,
    ):
        for r in range(SH):
            z_i8 = p1.tile([C, N], I8, tag="z_i8")
            # [tok, (t, c)] <- z_rows[r] viewed (t p) c -> p t c
            nc.gpsimd.dma_start(
                z_i8[:].rearrange("p (t c) -> p t c", t=T4),
                z_rows[r].rearrange("(t p) c -> p t c", p=C),
            )
            # int8 -> bf16; values are per-token scaled but LN whitening is
            # scale-invariant per token, so no dequant scale is needed
            z_sb = p1.tile([C, N], BF16, tag="z_sb")
            nc.vector.tensor_copy(z_sb[:], z_i8[:])
            mu4 = p1st.tile([C, T4], F32, tag="mu4")
            ssq4 = p1st.tile([C, T4], F32, tag="ssq4")
            sq_scr = p1st.tile([C, C], BF16, tag="sq_scr")
            for t in range(T4):
                zt = z_sb[:, t * C:(t + 1) * C]
                nc.vector.tensor_reduce(mu4[:, t:t + 1], zt,
                                        mybir.AxisListType.X, OP.add)
                nc.scalar.activation(sq_scr[:], zt, AF.Square,
                                     accum_out=ssq4[:, t:t + 1])
            nmu4 = p1st.tile([C, T4], F32, tag="nmu4")
            nc.vector.tensor_scalar_mul(nmu4[:], mu4[:], -1.0 / C)
            mu2 = p1st.tile([C, T4], F32, tag="mu2")
            nc.vector.tensor_tensor(mu2[:], nmu4[:], nmu4[:], OP.mult)
            var4 = p1st.tile([C, T4], F32, tag="var4")
            nc.vector.tensor_scalar_mul(var4[:], ssq4[:], 1.0 / C)
            var4b = p1st.tile([C, T4], F32, tag="var4b")
            nc.vector.tensor_tensor(var4b[:], var4[:], mu2[:], OP.subtract)
            std4 = p1st.tile([C, T4], F32, tag="std4")
            nc.scalar.activation(std4[:], var4b[:], AF.Sqrt,
                                 bias=cst['eps'][:])
            rstd4 = p1st.tile([C, T4], F32, tag="rstd4")
            nc.vector.reciprocal(rstd4[:], std4[:])

            zn_sb = p1.tile([C, N], BF16, tag="zn_sb")
            zT_ps = ps_zt.tile([C, N], BF16, tag="zT_ps")
            for t in range(T4):
                zt = z_sb[:, t * C:(t + 1) * C]
                znt = zn_sb[:, t * C:(t + 1) * C]
                nc.vector.tensor_scalar(
                    znt, zt, nmu4[:, t:t + 1], rstd4[:, t:t + 1],
                    OP.add, OP.mult)
                nc.tensor.transpose(zT_ps[:, t * C:(t + 1) * C], znt,
                                    cst['ident'][:])
            zT_sb = p1.tile([C, N], BF16, tag="zT_sb")
            nc.vector.tensor_copy(zT_sb[:], zT_ps[:])

            pap = ps_proj.tile([C, N], F32, tag="pap")
            pag = ps_proj.tile([C, N], F32, tag="pag")
            pbp = ps_proj.tile([C, N], F32, tag="pbp")
            pbg = ps_proj.tile([C, N], F32, tag="pbg")
            nc.tensor.matmul(pap[:], cst['wap'][:], zT_sb[:], start=True, stop=True)
            nc.tensor.matmul(pag[:], cst['wag'][:], zT_sb[:], start=True, stop=True)
            nc.tensor.matmul(pbp[:], cst['wbp'][:], zT_sb[:], start=True, stop=True)
            nc.tensor.matmul(pbg[:], cst['wbg'][:], zT_sb[:], start=True, stop=True)

            pa_sb = p1.tile([C, N], BF16, tag="pa_sb")
            pb_sb = p1.tile([C, N], BF16, tag="pb_sb")
            ga_sb = p1.tile([C, N], BF16, tag="ga_sb")
            gb_sb = p1.tile([C, N], BF16, tag="gb_sb")
            nc.vector.tensor_scalar_add(pa_sb[:], pap[:], cst['bap'][:])
            nc.scalar.activation(pb_sb[:], pbp[:], AF.Identity,
                                 bias=cst['bbp'][:])
            nc.scalar.activation(ga_sb[:], pag[:], AF.Sigmoid,
                                 bias=cst['bag'][:])
            nc.scalar.activation(gb_sb[:], pbg[:], AF.Sigmoid,
                                 bias=cst['bbg'][:])

            a1 = p1.tile([C, N], BF16, tag="a1")
            b1 = p1.tile([C, N], BF16, tag="b1")
            nc.vector.tensor_tensor(a1[:], pa_sb[:], ga_sb[:], OP.mult)
            nc.vector.tensor_tensor(b1[:], pb_sb[:], gb_sb[:], OP.mult)
            if MASK_MODE != 'skip':
                # mask row broadcast to 128 partitions via K=1 ones-matmul
                mask_ps = ps_mask.tile([C, N], F32, tag="mask_ps")
                nc.tensor.matmul(mask_ps[:], cst['ones1'][:],
                                 cst['mask'][:, r * N:(r + 1) * N],
                                 start=True, stop=True)
                mask_sb = p1.tile([C, N], BF16, tag="mask_sb")
                nc.scalar.copy(mask_sb[:], mask_ps[:])
                am = p1.tile([C, N], BF16, tag="am")
                bm = p1.tile([C, N], BF16, tag="bm")
                nc.vector.tensor_tensor(am[:], a1[:], mask_sb[:], OP.mult)
                nc.vector.tensor_tensor(bm[:], b1[:], mask_sb[:], OP.mult)
            else:
                am, bm = a1, b1
            nc.sync.dma_start(a_loc[:, r, :], am[:])
            nc.sync.dma_start(b_loc[:, r, :], bm[:])


def _phase2(tc, a_loc, b_all, o_mid):
    nc = tc.nc
    with (
        tc.tile_pool(name="p2a", bufs=2) as p2a,
        tc.tile_pool(name="p2b", bufs=2) as p2b,
        tc.tile_pool(name="p2o", bufs=3) as p2o,
        tc.tile_pool(name="ps_o", bufs=2, space="PSUM") as ps_o_pool,
    ):
        b_all_v = b_all[:].rearrange("(r c) k j -> r c k j", r=R)
        a_2d = a_loc[:].rearrange("c i k -> (c i) k")
        for oc in range(C // OCT):
            aT_t = []
            for q in range(NQ):
                at = p2a.tile([C, OCT * SH], BF16, tag=f"aT{q}")
                # src: a_loc[c-octet, :, k-chunk] as [(c i), k] 2D
                nc.sync.dma_start_transpose(
                    at[:],
                    a_2d[OCT * oc * SH:OCT * (oc + 1) * SH,
                         C * q:C * (q + 1)],
                )
                aT_t.append(at)
            RK = C // SH  # ranks per 128-row k-chunk
            b_t = []
            for q in range(NQ):
                bt = p2b.tile([C, OCT * N], BF16, tag=f"bT{q}")
                for rr in range(RK):
                    nc.sync.dma_start(
                        bt[rr * SH:(rr + 1) * SH, :].rearrange(
                            "k (c j) -> k c j", c=OCT),
                        b_all_v[RK * q + rr,
                                OCT * oc:OCT * (oc + 1), :, :].rearrange(
                            "c k j -> k c j"),
                    )
                b_t.append(bt)
            for ci in range(0, OCT, 2):
                o_sb = p2o.tile([SH, 2 * N], BF16, tag="o_sb")
                for cj in range(2):
                    ps_o = ps_o_pool.tile([SH, N], F32, tag="ps_o")
                    for q in range(NQ):
                        nc.tensor.matmul(
                            ps_o[:],
                            aT_t[q][:, (ci + cj) * SH:(ci + cj + 1) * SH],
                            b_t[q][:, (ci + cj) * N:(ci + cj + 1) * N],
                            start=(q == 0), stop=(q == NQ - 1))
                    nc.vector.tensor_copy(o_sb[:, cj * N:(cj + 1) * N],
                                          ps_o[:])
                c0 = OCT * oc + ci
                nc.sync.dma_start(
                    o_mid[c0:c0 + 2, :, :].rearrange("c k j -> k c j"),
                    o_sb[:].rearrange("k (c j) -> k c j", c=2))


def _phase3(tc, cst, o_mid, out_rows, scale_rows):
    nc = tc.nc
    with (
        tc.tile_pool(name="p3", bufs=3) as p3,
        tc.tile_pool(name="ps_f", bufs=4, space="PSUM") as ps_f_pool,
    ):
        for r in range(SH):
            oT_sb = p3.tile([C, N], BF16, tag="oT_sb")
            nc.sync.dma_start(oT_sb[:], o_mid[:, r, :])
            d32 = p3.tile([C, N], F32, tag="d32")
            am = p3.tile([C, T4], F32, tag="am")
            for t in range(T4):
                ps_f = ps_f_pool.tile([C, C], F32, tag="ps_f")
                nc.tensor.matmul(ps_f[:], oT_sb[:, t * C:(t + 1) * C],
                                 cst['wz'][:], start=True, stop=True)
                nc.vector.tensor_tensor(
                    d32[:, t * C:(t + 1) * C], ps_f[:],
                    cst['bzbc'][:], OP.add)
                nc.vector.tensor_reduce(
                    am[:, t:t + 1], d32[:, t * C:(t + 1) * C],
                    mybir.AxisListType.X, OP.max, apply_absolute_value=True)
            # per-token int8 quantization: q = d * 126/absmax, sc = absmax/126
            amc = p3.tile([C, T4], F32, tag="amc")
            nc.vector.tensor_scalar_max(amc[:], am[:], 1e-12)
            rs = p3.tile([C, T4], F32, tag="rs")
            nc.vector.reciprocal(rs[:], amc[:])
            rs126 = p3.tile([C, T4], F32, tag="rs126")
            nc.vector.tensor_scalar_mul(rs126[:], rs[:], 126.0)
            sc = p3.tile([C, T4], F32, tag="sc")
            nc.vector.tensor_scalar_mul(sc[:], amc[:], 1.0 / 126.0)
            q_i8 = p3.tile([C, N], I8, tag="q_i8")
            for t in range(T4):
                nc.vector.tensor_scalar_mul(
                    q_i8[:, t * C:(t + 1) * C], d32[:, t * C:(t + 1) * C],
                    rs126[:, t:t + 1])
            nc.sync.dma_start(
                out_rows[r].rearrange("(t p) c -> p t c", p=C),
                q_i8[:].rearrange("p (t c) -> p t c", t=T4))
            nc.sync.dma_start(
                scale_rows[r].rearrange("(t p) -> p t", p=C),
                sc[:])


def build():
    if 'nc' in _CACHE:
        return _CACHE['nc']
    nc = bacc.Bacc("TRN2", target_bir_lowering=False, debug=False,
                   num_devices=R)

    z_rows = nc.dram_tensor("z_rows", [SH, N, C], I8, kind="ExternalInput")
    mask_rows = nc.dram_tensor("mask_rows", [SH, N], F32, kind="ExternalInput")
    w_in = {}
    for nm in ("w_ap", "w_ag", "w_bp", "w_bg", "w_z"):
        w_in[nm] = nc.dram_tensor(nm, [C, C], BF16, kind="ExternalInput")
    b_in = {}
    for nm in ("b_ap", "b_ag", "b_bp", "b_bg"):
        b_in[nm] = nc.dram_tensor(nm, [C, 1], F32, kind="ExternalInput")
    bz_bc = nc.dram_tensor("bz_bc", [C, C], F32, kind="ExternalInput")
    out_rows = nc.dram_tensor("out_rows", [SH, N, C], I8,
                              kind="ExternalOutput")
    scale_rows = nc.dram_tensor("scale_rows", [SH, N], F32,
                                kind="ExternalOutput")

    with tile.TileContext(nc) as tc:
        with (
            tc.tile_pool(name="consts", bufs=1) as cpool,
            tc.tile_pool(name="dram", bufs=1, space="DRAM") as dram,
        ):
            cst = {}
            ident = cpool.tile([C, C], BF16)
            masks.make_identity(nc, ident[:])
            cst['ident'] = ident
            for nm, key in (("w_ap", 'wap'), ("w_ag", 'wag'),
                            ("w_bp", 'wbp'), ("w_bg", 'wbg'), ("w_z", 'wz')):
                t = cpool.tile([C, C], BF16, tag=f"c_{key}")
                nc.sync.dma_start(t[:], w_in[nm][:])
                cst[key] = t
            for nm, key in (("b_ap", 'bap'), ("b_ag", 'bag'),
                            ("b_bp", 'bbp'), ("b_bg", 'bbg')):
                t = cpool.tile([C, 1], F32, tag=f"c_{key}")
                nc.sync.dma_start(t[:], b_in[nm][:])
                cst[key] = t
            bzbc = cpool.tile([C, C], F32)
            nc.sync.dma_start(bzbc[:], bz_bc[:])
            cst['bzbc'] = bzbc
            # whole mask shard on partition 0, bf16 (for K=1 broadcast matmuls)
            mask_p0 = cpool.tile([1, SH * N], BF16)
            nc.gpsimd.dma_start(mask_p0[:],
                                mask_rows[:].rearrange("r n -> (r n)")
                                .unsqueeze(0))
            cst['mask'] = mask_p0
            ones1 = cpool.tile([1, C], BF16)
            nc.vector.memset(ones1[:], 1.0)
            cst['ones1'] = ones1
            eps = cpool.tile([C, 1], F32)
            nc.vector.memset(eps[:], 1e-5)
            cst['eps'] = eps

            a_loc = dram.tile([C, SH, N], BF16)      # [c, i_loc, k]
            b_loc = dram.tile([C, SH, N], BF16)      # [c, k_loc, j]
            b_all = dram.tile([R * C, SH, N], BF16,
                              addr_space="Shared")   # [(rank c), k_loc, j]
            o_mid = dram.tile([C, SH, N], BF16)      # [c, i_loc, j]

            _phase1(tc, cst, z_rows, a_loc, b_loc)
            nc.gpsimd.collective_compute(
                "AllGather", OP.bypass,
                replica_groups=[list(range(R))],
                ins=[b_loc[:].opt()],
                outs=[b_all[:].opt()],
            )
            _phase2(tc, a_loc, b_all, o_mid)
            _phase3(tc, cst, o_mid, out_rows, scale_rows)

    nc.compile()
    _CACHE['nc'] = nc
    return nc


def _get_runner():
    """Build (once) a cached jitted shard_map callable around the bass NEFF.

    Mirrors concourse.bass2jax.run_bass_via_pjrt's multi-core branch, except:
      - output zero-buffers are created on-device inside the body (the stock
        path uploads host zeros and donates them -- 67MB/call over the tunnel)
      - the jitted function is built once and cached, so repeat calls skip
        retracing/compilation and go straight to dispatch
    """
    if 'runner' in _CACHE:
        return _CACHE['runner']

    import jax
    import jax.numpy as jnp
    from jax.experimental.shard_map import shard_map
    from jax.sharding import Mesh, NamedSharding, PartitionSpec

    nc = build()
    bass2jax.install_neuronx_cc_hook()
    assert nc.dbg_addr is None

    partition_name = (nc.partition_id_tensor.name
                      if nc.partition_id_tensor else None)
    in_names = []
    out_names = []
    out_avals = []
    for alloc in nc.m.functions[0].allocations:
        if not isinstance(alloc, mybir.MemoryLocationSet):
            continue
        name = alloc.memorylocations[0].name
        if alloc.kind == "ExternalInput":
            if name != partition_name:
                in_names.append(name)
        elif alloc.kind == "ExternalOutput":
            out_names.append(name)
            out_avals.append(jax.core.ShapedArray(
                tuple(alloc.tensor_shape), mybir.dt.np(alloc.dtype)))
    n_params = len(in_names)
    n_outs = len(out_names)
    all_in_names = list(in_names) + list(out_names)
    if partition_name is not None:
        all_in_names.append(partition_name)

    def _body(*args):
        # args: kernel inputs followed by (device-resident) output zero bufs;
        # the hook requires every custom-call operand to be a jit parameter
        operands = list(args)
        if partition_name is not None:
            operands.append(bass2jax.partition_id_tensor())
        outs = bass2jax._bass_exec_p.bind(
            *operands,
            out_avals=tuple(out_avals),
            in_names=tuple(all_in_names),
            out_names=tuple(out_names),
            lowering_input_output_aliases=(),
            sim_require_finite=True,
            sim_require_nnan=True,
            nc=nc,
        )
        return tuple(outs)

    devices = jax.devices()[:R]
    assert len(devices) == R, f"need {R} devices, have {len(jax.devices())}"
    mesh = Mesh(np.asarray(devices), ("core",))
    in_specs = (PartitionSpec("core"),) * (n_params + n_outs)
    out_specs = (PartitionSpec("core"),) * n_outs
    sharded = jax.jit(shard_map(
        _body, mesh=mesh, in_specs=in_specs,
        out_specs=out_specs, check_rep=False))

    # output zero buffers: built on-device once, reused every call (the
    # kernel writes every output element, so stale contents are fine)
    shardings = [NamedSharding(mesh, PartitionSpec("core"))] * n_outs
    global_shapes = [(R * a.shape[0],) + tuple(a.shape[1:]) for a in out_avals]

    def _mk_zeros():
        return tuple(jnp.zeros(s, a.dtype)
                     for s, a in zip(global_shapes, out_avals))

    zeros_dev = jax.jit(_mk_zeros, out_shardings=tuple(shardings))()

    _CACHE['runner'] = (sharded, in_names, out_names, zeros_dev,
                        (devices, shardings[0]))
    return _CACHE['runner']


def _host_fns():
    """Fused single-pass CPU helpers (XLA fuses the multi-op numpy chains),
    operating on one 64-row shard at a time so host work can pipeline with
    tunnel transfers."""
    if 'hostfns' not in _CACHE:
        import jax
        import jax.numpy as jnp
        from functools import partial

        @partial(jax.jit, backend='cpu')
        def quant(zf):
            am = jnp.max(jnp.abs(zf), axis=-1)
            rs = jnp.float32(126.0) / jnp.maximum(am, jnp.float32(1e-12))
            return jnp.rint(zf * rs[:, :, None]).astype(jnp.int8)

        @partial(jax.jit, backend='cpu')
        def dequant(q, sc, zf):
            return zf + q.astype(jnp.float32) * sc[:, :, None]

        _CACHE['hostfns'] = (quant, dequant)
    return _CACHE['hostfns']


def kernel(z, mask, ln_w, ln_b, W_ap, b_ap, W_ag, b_ag, W_bp, b_bp,
           W_bg, b_bg, W_z, b_z):
    z = np.asarray(z, dtype=np.float32)
    mask = np.asarray(mask, dtype=np.float32)
    ln_w = np.asarray(ln_w, np.float32)
    ln_b = np.asarray(ln_b, np.float32)
    bf = ml_dtypes.bfloat16

    def fold_w(W):
        return np.ascontiguousarray((ln_w[:, None] * np.asarray(W, np.float32))
                                    .astype(bf))

    def fold_b(b, W):
        return np.ascontiguousarray(
            (np.asarray(b, np.float32) + ln_b @ np.asarray(W, np.float32))
            .reshape(C, 1))

    weights = dict(
        w_ap=fold_w(W_ap), w_ag=fold_w(W_ag),
        w_bp=fold_w(W_bp), w_bg=fold_w(W_bg),
        b_ap=fold_b(b_ap, W_ap), b_ag=fold_b(b_ag, W_ag),
        b_bp=fold_b(b_bp, W_bp), b_bg=fold_b(b_bg, W_bg),
        w_z=np.ascontiguousarray(np.asarray(W_z, np.float32).astype(bf)),
        bz_bc=np.ascontiguousarray(
            np.broadcast_to(np.asarray(b_z, np.float32), (C, C))),
    )

    quant, dequant = _host_fns()
    zf = z.reshape(N, N, C)
    mask_full = mask.reshape(N, N)             # global (R*SH, N) concat

    sharded, in_names, out_names, zeros_dev, mesh_info = _get_runner()
    import jax
    import concurrent.futures as cf
    devices, z_sharding = mesh_info

    # per-token int8 quantization of z (LN on device is scale-invariant, so
    # the scale never needs to leave the host), one shard at a time with the
    # per-device upload issued immediately so quantize overlaps the tunnel
    parts = []
    for i in range(R):
        q_i = quant(zf[i * SH:(i + 1) * SH])
        parts.append(jax.device_put(q_i, devices[i]))
    z_q = jax.make_array_from_single_device_arrays(
        (N, N, C), z_sharding, parts)

    # global concat-along-axis-0 arrays, in in_names order
    global_ins = {
        'z_rows': z_q,
        'mask_rows': mask_full,
    }
    for k, v in weights.items():
        global_ins[k] = np.ascontiguousarray(
            np.tile(v, (R,) + (1,) * (v.ndim - 1)))

    args = [global_ins[name] for name in in_names]
    outs = sharded(*args, *zeros_dev)
    q_g = outs[out_names.index("out_rows")]        # (N, N, C) int8 sharded
    sc_g = outs[out_names.index("scale_rows")]     # (N, N) f32 sharded

    # fetch output shards over the tunnel while dequantizing completed ones
    out = np.empty((N, N, C), np.float32)
    q_shards = sorted(q_g.addressable_shards, key=lambda s: s.index[0].start)
    sc_shards = sorted(sc_g.addressable_shards, key=lambda s: s.index[0].start)
    with cf.ThreadPoolExecutor(8) as ex:
        q_futs = [ex.submit(lambda s=s: np.asarray(s.data)) for s in q_shards]
        sc_futs = [ex.submit(lambda s=s: np.asarray(s.data)) for s in sc_shards]
        for i in range(R):
            sl = slice(i * SH, (i + 1) * SH)
            out[sl] = dequant(q_futs[i].result(), sc_futs[i].result(), zf[sl])
    return out.reshape(1, N, N, C)
